# revision 21
# baseline (speedup 1.0000x reference)
"""Trainium2 Bass kernel for nn_FDConv (per-sample frequency-domain-synthesized
3x3 grouped conv).

Strategy (data-parallel over batch, 1 sample per NeuronCore):
  - host: permute dft_weight into dense half-spectrum layout (pure gather),
    precompute DFT basis matrices (incl. a negated copy of C so no on-device
    negations are needed), stage x as bf16 in a padded parity-split layout.
  - device per core:
      att = sigmoid(logits), broadcast via K=1 matmul
      M   = per-sample mixed spectrum: 4 wide DVE FMA ops over the 4 kernels
            (kernel-major layout -> one [96, 388] op per kernel, final op
            fuses the bf16 cast)
      GT  = M^T . [C | S | -C]  (stage-1 iFFT along axis 0, 8 matmuls)
      T[which,dx] = A_dx^T . GT_re - B_dx^T . GT_im  (stage-2 irfft + reshape
            to six 128x128 conv weight mats)
      conv: x bf16 with even rows on partitions 0-63 and odd rows on 64-127;
            two output row pairs per matmul group: six K=128/M=128/N=512
            matmuls (75% PE utilization bound is structural for this shape)
            accumulated in 8 preallocated PSUM banks.
  - outputs copied PSUM->SBUF as bf16 (DVE/ACT) and DMA'd back bf16 with few,
    large descriptors (store issue cost ~0.7us each dominates the tail);
    host reassembles the row interleave in fp32.
"""

import numpy as np
import ml_dtypes

import concourse.bass as bass
import concourse.bacc as bacc
import concourse.tile as tile
import concourse.mybir as mybir
from concourse.bass_utils import run_bass_kernel_spmd

F32 = mybir.dt.float32
BF16 = mybir.dt.bfloat16
MULT = mybir.AluOpType.mult
ADD = mybir.AluOpType.add

B, CIN, COUT, KS = 8, 64, 64, 3
H, W = 256, 256
KNUM = 4
D1, D2 = COUT * KS, CIN * KS          # 192, 192
D2R = D2 // 2 + 1                     # 97
NF = D1 * D2R                         # 18624

NPAIR = 128          # output row pairs (2u+1, 2u+2), u = -1..127
SLOT = W + 2         # 258: [pad, 256 cols, pad] per row-slot
# x chunk slot boundaries: small first chunk so the first conv blocks' slots
# land early (startup is early-phase DMA-bandwidth bound)
XB = [0, 8, 24, 32, 48, 64, 80, 96, 112, 128]
NCH = len(XB) - 1


def _host_constants():
    fh = np.fft.fftfreq(D1)
    fw = np.fft.rfftfreq(D2)
    dist = np.sqrt(fh[:, None] ** 2 + fw[None, :] ** 2)
    idx = np.argsort(dist.ravel(), kind='stable')
    FH = (idx // D2R).astype(np.int64)
    FW = (idx % D2R).astype(np.int64)
    perm = FH * D2R + FW
    inv = np.empty(NF, dtype=np.int64)
    inv[perm] = np.arange(NF)

    hh = np.arange(D1)
    ang = 2.0 * np.pi * np.outer(hh, hh) / D1
    # att scale 2/KNUM = 0.5 folded into the stage-1 basis
    Cb = (np.cos(ang) * (0.5 / D1)).astype(np.float32)
    Sb = (np.sin(ang) * (0.5 / D1)).astype(np.float32)
    C2 = np.concatenate([Cb[:96], Cb[96:]], axis=1)           # [96, 384]
    S2 = np.concatenate([Sb[:96], Sb[96:]], axis=1)           # [96, 384]
    cs = np.concatenate([C2, S2], axis=1).astype(ml_dtypes.bfloat16)

    w_ = np.arange(D2R)
    n_ = np.arange(D2)
    alpha = np.full(D2R, 2.0); alpha[0] = 1.0; alpha[-1] = 1.0
    beta = np.full(D2R, 2.0); beta[0] = 0.0; beta[-1] = 0.0
    ang2 = 2.0 * np.pi * np.outer(w_, n_) / D2
    A = (alpha[:, None] * np.cos(ang2) / D2).astype(np.float32)   # [97, 192]
    Bm = (beta[:, None] * np.sin(ang2) / D2).astype(np.float32)
    ab = np.concatenate(
        [A[:, dx::3] for dx in range(3)] + [-Bm[:, dx::3] for dx in range(3)],
        axis=1,
    ).astype(ml_dtypes.bfloat16)                               # [97, 384]
    return inv, cs, ab


_INV, _CS, _AB = _host_constants()

# (which, dx) order used in the conv weight loop; t_sb index = 2*dx + which
_WSEQ = [(0, 0), (1, 0), (0, 1), (1, 1), (0, 2), (1, 2)]
# valid quadrants (J, r, c0) per `which` (c0 = J - r + 2*which)
_QUADS = {
    0: [(0, 0, 0), (1, 0, 1), (1, 1, 0)],
    1: [(0, 0, 2), (0, 1, 1), (1, 1, 2)],
}
_ZQUAD = {0: (0, 1), 1: (1, 0)}  # zero quadrant (J, r)


def _emit_kernel(tc):
    nc = tc.nc
    from contextlib import ExitStack

    # x: [parity, cin, slot, 258] bf16, host-padded (col 0 and 257 are zeros)
    x_in = nc.dram_tensor("x_in", [2, CIN, NPAIR, SLOT], BF16,
                          kind="ExternalInput").ap()
    lg_in = nc.dram_tensor("lg_in", [1, KNUM], F32, kind="ExternalInput").ap()
    # spectrum, kernel-major: cols k*(4*97) + q*97 + w with quadrant
    # q in {0:(re,h0), 1:(re,h1), 2:(imneg,h0), 3:(imneg,h1)}
    dd_in = nc.dram_tensor("dd_in", [96, KNUM * 4 * D2R], BF16,
                           kind="ExternalInput").ap()
    cs_in = nc.dram_tensor("cs_in", [96, 4 * D1], BF16,
                           kind="ExternalInput").ap()
    ab_in = nc.dram_tensor("ab_in", [D2R, 6 * 64], BF16,
                           kind="ExternalInput").ap()
    # device-side output layout: plane r=0 slot s = row 2s-1, plane r=1 slot
    # s = row 2s. Keeps every store descriptor contiguous per partition;
    # host reassembles rows.
    out = nc.dram_tensor("out", [2, COUT, NPAIR + 1, W], BF16,
                         kind="ExternalOutput").ap()

    KW = 4 * D2R  # 388, one kernel's width in dd

    with ExitStack() as ctx:
        cpool = ctx.enter_context(tc.tile_pool(name="cpool", bufs=1))
        xbpool = ctx.enter_context(tc.tile_pool(name="xbpool", bufs=1))
        spool = ctx.enter_context(tc.tile_pool(name="spool", bufs=3))

        # ---- latency-critical small loads. DMA transfers complete roughly
        # in global issue order at ~135 GB/s early on, so the T-synthesis
        # chain loads must be the first issues on every engine; everything
        # else is gated behind progress markers of the T chain.
        l_sb = cpool.tile([1, KNUM], F32, name="l_sb")
        dd_sb = cpool.tile([96, KNUM * KW], BF16, name="dd_sb")
        cs_sb = cpool.tile([96, 4 * D1], BF16, name="cs_sb")
        nc2_sb = cpool.tile([96, 2 * D1], BF16, name="nc2_sb")
        ab_sb = cpool.tile([D2R, 6 * 64], BF16, name="ab_sb")

        nc.sync.dma_start(out=l_sb[:], in_=lg_in)
        nc.sync.dma_start(out=dd_sb[:, 0:KW], in_=dd_in[:, 0:KW])           # k0
        nc.gpsimd.dma_start(out=dd_sb[:, KW:2 * KW], in_=dd_in[:, KW:2 * KW])
        nc.sync.dma_start(out=dd_sb[:, 2 * KW:3 * KW],
                          in_=dd_in[:, 2 * KW:3 * KW])                      # k2
        nc.gpsimd.dma_start(out=dd_sb[:, 3 * KW:4 * KW],
                            in_=dd_in[:, 3 * KW:4 * KW])                    # k3
        nc.sync.dma_start(out=cs_sb[:, 0:2 * D1], in_=cs_in[:, 0:2 * D1])   # C2
        nc.gpsimd.dma_start(out=cs_sb[:, 2 * D1:4 * D1],
                            in_=cs_in[:, 2 * D1:4 * D1])                    # S2
        # ab is small and gates stage 2; keep it inside the first cohort
        nc.gpsimd.dma_start(out=ab_sb[:], in_=ab_in)

        # sigmoid first in the ACT stream (its act table loads before it)
        sig_sb = cpool.tile([1, KNUM], F32, name="sig_sb")
        sig_i = nc.scalar.activation(sig_sb[:], l_sb[:],
                                     mybir.ActivationFunctionType.Sigmoid)
        ones_sb = cpool.tile([1, 128], F32, name="ones_sb")
        nc.vector.memset(ones_sb[:], 1.0)

        # ---- per-sample spectrum mixing: 4 wide DVE FMAs (fp32 chain, the
        # last op writes bf16). Layout: [re_h0 | re_h1 | imneg_h0 | imneg_h1].
        mm_bf = cpool.tile([96, KW], BF16, name="mm_bf")
        macc = cpool.tile([96, KW], F32, name="macc")

        gtre_sb = cpool.tile([D2R, D1 + 2], BF16, name="gtre_sb")
        gtim_sb = cpool.tile([D2R, D1 + 2], BF16, name="gtim_sb")
        t_sb = [cpool.tile([128, 128], BF16, name=f"t_sb_{i}") for i in range(6)]

        dftps_cm = tc.tile_pool(name="dftps", bufs=1, space="PSUM")
        dpool = dftps_cm.__enter__()
        # att broadcast: [128, 4] = ones.T @ sig
        att_ps = dpool.tile([128, KNUM], F32, name="att_ps")
        nc.tensor.matmul(att_ps[:], ones_sb[:], sig_sb[:], start=True, stop=True)
        att_sb = cpool.tile([96, KNUM], F32, name="att_sb")
        att_cp = nc.vector.tensor_copy(att_sb[:], att_ps[0:96, :])

        nc.vector.tensor_scalar_mul(macc[:], dd_sb[:, 0:KW], att_sb[:, 0:1])
        fma1 = nc.vector.scalar_tensor_tensor(
            macc[:], dd_sb[:, KW:2 * KW], att_sb[:, 1:2], macc[:], MULT, ADD)
        nc.vector.scalar_tensor_tensor(
            macc[:], dd_sb[:, 2 * KW:3 * KW], att_sb[:, 2:3], macc[:],
            MULT, ADD)
        fma3 = nc.vector.scalar_tensor_tensor(
            mm_bf[:], dd_sb[:, 3 * KW:4 * KW],
            att_sb[:, 3:4], macc[:], MULT, ADD)

        # ---- x chunk loads: slot t holds rows (2t, 2t+1). Issue cohorts:
        # chunk0 slots 0-7 behind att_cp, chunk0 tail + chunk1 first half
        # behind the k1 mixing FMA, the rest behind the full dft chain.
        xch = []
        xdmas = []
        gated = {id(att_cp): [], id(fma1): [], id(fma3): []}
        for c in range(NCH):
            # chunks overlap one slot so 2-pair windows never cross a tile
            # boundary; the last chunk ends at slot 127
            nsl = XB[c + 1] - XB[c] + (1 if c + 1 < NCH else 0)
            xb = xbpool.tile([128, nsl * SLOT], BF16, name=f"xb{c}")
            if c == 0:
                engs = [nc.scalar, nc.sync, nc.scalar, nc.gpsimd,
                        nc.sync, nc.gpsimd, nc.sync, nc.gpsimd]
                ranges = ((0, 7), (7, nsl))
                gates = (att_cp, fma1)
            elif c == 1:
                engs = [nc.sync, nc.gpsimd, nc.scalar, nc.scalar,
                        nc.gpsimd, nc.sync, nc.gpsimd, nc.sync]
                ranges = ((0, 9), (9, nsl))
                gates = (fma3, fma3)
            else:
                engs = None
            if c < 2:
                ei = 0
                for (s0, s1), gate in zip(ranges, gates):
                    for par in range(2):
                        for p0, p1 in ((0, 32), (32, 64)):
                            di = engs[ei].dma_start(
                                out=xb[64 * par + p0: 64 * par + p1,
                                       s0 * SLOT:s1 * SLOT],
                                in_=x_in[par, p0:p1,
                                         XB[c] + s0:XB[c] + s1, :])
                            if engs[ei] is nc.scalar:
                                # keep the ACT stream clear for the sigmoid
                                bass._add_dep_helper(
                                    di.ins, sig_i.ins,
                                    reason="sigmoid before scalar descgen")
                            if gate == 'tcopy':
                                xdmas.append(di)
                            else:
                                gated[id(gate)].append(di)
                            ei += 1
            else:
                # alternate gpsimd/sync so neither queue falls behind the
                # conv's slot consumption mid-stream
                for par in range(2):
                    for pi, (p0, p1) in enumerate(((0, 32), (32, 64))):
                        eng = nc.gpsimd if pi == 0 else nc.sync
                        di = eng.dma_start(
                            out=xb[64 * par + p0: 64 * par + p1, 0:nsl * SLOT],
                            in_=x_in[par, p0:p1, XB[c]:XB[c] + nsl, :])
                        xdmas.append(di)
            xch.append(xb)
        for di in gated[id(att_cp)]:
            bass._add_dep_helper(di.ins, att_cp.ins,
                                 reason="bulk x yields HBM to T-chain loads")
        for di in gated[id(fma1)]:
            bass._add_dep_helper(di.ins, fma1.ins,
                                 reason="bulk x yields HBM to T-chain loads")
        for di in gated[id(fma3)]:
            bass._add_dep_helper(di.ins, fma3.ins,
                                 reason="bulk x yields HBM to T-chain loads")

        # -C2 basis computed on ACT (saves 74KB of critical-path loads);
        # emitted after scalar's gated x pieces so it doesn't block them
        nc.scalar.mul(nc2_sb[:], cs_sb[:, 0:2 * D1], -1.0)

        def slot_rhs(s, dx, npair=1):
            # [128, npair, W] window starting at slot s (npair<=2; both slots
            # live in one chunk thanks to the 1-slot overlap)
            for c in range(NCH):
                if s < XB[c + 1]:
                    break
            loc = s - XB[c]
            if npair == 1:
                return xch[c][:, loc * SLOT + dx: loc * SLOT + dx + W]
            v = xch[c].rearrange("p (t s) -> p t s", s=SLOT)
            return v[:, loc:loc + npair, dx:dx + W]

        if True:
            def mview(q):
                return mm_bf[:, q * D2R:(q + 1) * D2R]

            def csv(which, h):
                if which == 2:
                    return nc2_sb[:, h * D1:(h + 1) * D1]
                return cs_sb[:, which * 2 * D1 + h * D1:
                             which * 2 * D1 + (h + 1) * D1]

            # ---- stage 1: GT = M^T . [C|S|-C], contraction over h halves
            gtre_ps = dpool.tile([D2R, D1], F32, name="gtre_ps")
            gtim_ps = dpool.tile([D2R, D1], F32, name="gtim_ps")
            # gtre = mre.C + mineg.S ; gtim = mre.S + mineg.(-C)
            nc.tensor.matmul(gtre_ps[:], mview(0), csv(0, 0),
                             start=True, stop=False)
            nc.tensor.matmul(gtim_ps[:], mview(0), csv(1, 0),
                             start=True, stop=False)
            nc.tensor.matmul(gtre_ps[:], mview(1), csv(0, 1),
                             start=False, stop=False)
            nc.tensor.matmul(gtim_ps[:], mview(1), csv(1, 1),
                             start=False, stop=False)
            nc.tensor.matmul(gtre_ps[:], mview(2), csv(1, 0),
                             start=False, stop=False)
            nc.tensor.matmul(gtim_ps[:], mview(2), csv(2, 0),
                             start=False, stop=False)
            nc.tensor.matmul(gtre_ps[:], mview(3), csv(1, 1),
                             start=False, stop=True)
            nc.tensor.matmul(gtim_ps[:], mview(3), csv(2, 1),
                             start=False, stop=True)
            nc.vector.tensor_copy(gtre_sb[:, 0:D1], gtre_ps[:])
            nc.vector.tensor_copy(gtim_sb[:, 0:D1], gtim_ps[:])

            # ---- stage 2: six conv weight matrices T[(ci,j),(co,r)]
            def gview(g, c0):
                return g[:, c0:c0 + D1].rearrange(
                    "w (co th) -> w co th", th=3)[:, :, 0:1]

            t_copy = None
            for i, (which, dx) in enumerate(_WSEQ):
                t_ps = dpool.tile([128, 128], F32, name="t_ps", bufs=3)
                zj, zr = _ZQUAD[which]
                nc.vector.memset(t_ps[64 * zj:64 * zj + 64, 64 * zr:64 * zr + 64], 0.0)
                for (J, r, c0) in _QUADS[which]:
                    o = t_ps[64 * J:64 * J + 64, 64 * r:64 * r + 64]
                    nc.tensor.matmul(o, ab_sb[:, dx * 64:(dx + 1) * 64],
                                     gview(gtre_sb, c0), start=True, stop=False)
                    nc.tensor.matmul(o, ab_sb[:, (3 + dx) * 64:(4 + dx) * 64],
                                     gview(gtim_sb, c0), start=False, stop=True)
                t_copy = nc.vector.tensor_copy(t_sb[2 * dx + which][:], t_ps[:])
        dftps_cm.__exit__(None, None, None)
        # late x chunks wait for the dft chain to clear the HBM/SDMA path
        for di in xdmas:
            bass._add_dep_helper(di.ins, t_copy.ins,
                                 reason="late x chunks yield HBM to dft chain")

        # ---- conv over row pairs
        # staging groups over pair slots s = u+1 in [0, 129): big early, small
        # at the end so the last stores drain quickly
        gsizes = [16] * 6 + [8, 8, 8, 4, 2, 1, 1, 1]
        gstart = np.cumsum([0] + gsizes).tolist()

        def group_of(s):
            for gi in range(len(gsizes)):
                if s < gstart[gi + 1]:
                    return gi, s - gstart[gi]
            raise AssertionError

        # units: (-1,) special, (0,1), (2,3), ..., (124,125), (126,), (127,)
        units = [(-1,)] + [(u, u + 1) for u in range(0, 126, 2)] + [(126,), (127,)]

        with tc.tile_pool(name="convps", bufs=1, space="PSUM") as cps:
            # 8 preallocated PSUM bank tiles reused round-robin: identical
            # schedule to per-block allocation with less pool bookkeeping.
            ptiles = [cps.tile([128, 2 * W], F32, name=f"pp{i}")
                      for i in range(8)]
            pidx = [0]
            staging = {}

            def get_staging(gi):
                if gi not in staging:
                    if gi >= 9:
                        # small late groups get dedicated slots so the final
                        # copies never wait on store completions
                        staging[gi] = spool.tile(
                            [128, gsizes[gi] * W], BF16,
                            name=f"staging_l{gi}", bufs=1)
                    else:
                        staging[gi] = spool.tile(
                            [128, gsizes[gi] * W], BF16,
                            name=f"staging{gsizes[gi]}")
                return staging[gi]

            def unit_mms(un):
                L = []
                for wh, dx in _WSEQ:
                    if wh == 0 and un[0] < 0:
                        continue
                    if wh == 1 and un[0] > 126:
                        continue
                    L.append((wh, dx))
                return L

            def emit_block(uns):
                tiles = {}
                for un in uns:
                    tiles[un] = ptiles[pidx[0] % 8][:, 0:len(un) * W]
                    pidx[0] += 1
                plan = {un: unit_mms(un) for un in uns}
                for k, (wh, dx) in enumerate(_WSEQ):
                    for un in uns:
                        if (wh, dx) not in plan[un]:
                            continue
                        i = plan[un].index((wh, dx))
                        rhs = slot_rhs(un[0] + (0 if wh == 0 else 1), dx,
                                       len(un))
                        nc.tensor.matmul(
                            tiles[un][:], t_sb[2 * dx + wh][:], rhs,
                            start=(i == 0), stop=(i == len(plan[un]) - 1),
                            skip_group_check=True)
                for un in uns:
                    for j, u in enumerate(un):
                        gi, si = group_of(u + 1)
                        st = get_staging(gi)[:, si * W:(si + 1) * W]
                        src = tiles[un][:, j * W:(j + 1) * W]
                        if u == -1:
                            nc.scalar.copy(st[64:128, :], src[64:128, :])
                        elif u == 127:
                            nc.scalar.copy(st[0:64, :], src[0:64, :])
                        elif (j == 0 and len(un) == 2 and
                              group_of(un[1] + 1)[0] == gi):
                            # both halves land in the same staging tile: one
                            # wide copy, alternating engines per unit
                            st2 = get_staging(gi)[:, si * W:(si + 2) * W]
                            if (u // 2) % 2 == 0:
                                nc.vector.tensor_copy(st2, tiles[un][:])
                            else:
                                nc.scalar.copy(st2, tiles[un][:])
                            break
                        elif u % 2 == 0:
                            nc.vector.tensor_copy(st, src)
                        else:
                            nc.scalar.copy(st, src)
                    for u in un:
                        gi, si = group_of(u + 1)
                        if si == gsizes[gi] - 1:
                            emit_stores(gi)

            store_cnt = [0]

            def store_dma(dst, src, late=False):
                # round-robin store engines; sync carries half the x loads,
                # so early stores go on scalar/gpsimd
                if late == 0:
                    engs = [nc.scalar, nc.gpsimd]
                elif late == 1:
                    engs = [nc.scalar, nc.sync, nc.gpsimd]
                else:
                    engs = [nc.sync, nc.scalar, nc.gpsimd]
                eng = engs[store_cnt[0] % len(engs)]
                store_cnt[0] += 1
                eng.dma_start(out=dst, in_=src)

            def emit_stores(gi):
                stg = staging.pop(gi)
                s0, s1 = gstart[gi], gstart[gi + 1]
                late = 0 if gi < 4 else (1 if gi < 9 else 2)
                sv = stg.rearrange("p (g w) -> p g w", w=W)
                if gi == 0:
                    # row 0 from pair u=-1 -> plane 1, slot 0
                    store_dma(out[1, :, 0:1, :], sv[64:128, 0:1, :])
                # full pairs in this group: slots max(s0,1) .. min(s1,128)-1
                fa, fb = max(s0, 1), min(s1, 128)
                # small transfers near the end so the post-conv drain is short
                run = 8 if gi < 4 else 4
                va = fa
                while va < fb:
                    vb = min(fb, va + run)
                    G = vb - va
                    store_dma(out[0, :, va:vb, :],
                              sv[0:64, va - s0:va - s0 + G, :], late)
                    store_dma(out[1, :, va:vb, :],
                              sv[64:128, va - s0:va - s0 + G, :], late)
                    va = vb
                if s1 == 129:
                    # row 255 from pair u=127 -> plane 0, slot 128
                    store_dma(out[0, :, NPAIR:NPAIR + 1, :],
                              sv[0:64, 128 - s0:129 - s0, :], True)

            # blocks of up to 4 units
            ui = 0
            while ui < len(units):
                emit_block(units[ui:ui + 4])
                ui += 4


_NC_CACHE = None


def _build_nc():
    global _NC_CACHE
    if _NC_CACHE is None:
        nc = bacc.Bacc("TRN2", target_bir_lowering=False, debug=False,
                       num_devices=B)
        with tile.TileContext(nc) as tc:
            _emit_kernel(tc)
        nc.compile()
        _NC_CACHE = nc
    return _NC_CACHE


def _in_maps(x, k_att_logits, dft_weight):
    x = np.asarray(x, dtype=np.float32)
    lg = np.asarray(k_att_logits, dtype=np.float32)
    dw = np.asarray(dft_weight, dtype=np.float32)

    # x -> bf16, parity-split rows, host-inserted zero pad columns
    xp = np.zeros((B, 2, CIN, NPAIR, SLOT), dtype=ml_dtypes.bfloat16)
    xv = x.reshape(B, CIN, NPAIR, 2, W).transpose(0, 3, 1, 2, 4)  # [b,j,c,t,w]
    xp[:, :, :, :, 1:1 + W] = xv.astype(ml_dtypes.bfloat16)

    # host-side gather: dense half-spectrum layout, kernel-major quadrants
    dftP = dw[:, _INV, :].reshape(KNUM, 2, 96, D2R, 2)   # [k, half, p, w, c]
    re = dftP[..., 0]             # [k, half, p, w]
    imneg = -dftP[..., 1]
    quad = np.stack([re[:, 0], re[:, 1], imneg[:, 0], imneg[:, 1]],
                    axis=1)       # [k, q, p, w]
    dd = np.ascontiguousarray(
        quad.transpose(2, 0, 1, 3).reshape(96, KNUM * 4 * D2R)
    ).astype(ml_dtypes.bfloat16)

    maps = []
    for b in range(B):
        maps.append({
            "x_in": np.ascontiguousarray(xp[b]),
            "lg_in": lg[b:b + 1],
            "dd_in": dd,
            "cs_in": _CS,
            "ab_in": _AB,
        })
    return maps


def _execute(x, k_att_logits, dft_weight, trace=False, **trace_kwargs):
    nc = _build_nc()
    res = run_bass_kernel_spmd(
        nc, _in_maps(x, k_att_logits, dft_weight),
        core_ids=list(range(B)), trace=trace, **trace_kwargs)
    out = np.empty((B, COUT, H, W), dtype=np.float32)
    for b in range(B):
        dev = res.results[b]["out"]      # [2, COUT, NPAIR+1, W] bf16
        out[b, :, 1::2, :] = dev[0, :, 1:NPAIR + 1, :].astype(np.float32)
        out[b, :, 0::2, :] = dev[1, :, 0:NPAIR, :].astype(np.float32)
    return out, res


def kernel(x, k_att_logits, dft_weight):
    out, _ = _execute(x, k_att_logits, dft_weight)
    return out.astype(np.float32)


# revision 22
# speedup vs baseline: 1.0169x; 1.0169x over previous
"""Trainium2 Bass kernel for nn_FDConv (per-sample frequency-domain-synthesized
3x3 grouped conv).

Strategy (data-parallel over batch, 1 sample per NeuronCore):
  - host: permute dft_weight into dense half-spectrum layout (pure gather),
    precompute DFT basis matrices (incl. a negated copy of C so no on-device
    negations are needed), stage x as bf16 in a padded parity-split layout.
  - device per core:
      att = sigmoid(logits), broadcast via K=1 matmul
      M   = per-sample mixed spectrum: 4 wide DVE FMA ops over the 4 kernels
            (kernel-major layout -> one [96, 388] op per kernel, final op
            fuses the bf16 cast)
      GT  = M^T . [C | S | -C]  (stage-1 iFFT along axis 0, 8 matmuls)
      T[which,dx] = A_dx^T . GT_re - B_dx^T . GT_im  (stage-2 irfft + reshape
            to six 128x128 conv weight mats)
      conv: x bf16 with even rows on partitions 0-63 and odd rows on 64-127;
            two output row pairs per matmul group: six K=128/M=128/N=512
            matmuls (75% PE utilization bound is structural for this shape)
            accumulated in 8 preallocated PSUM banks.
  - outputs copied PSUM->SBUF as bf16 (DVE/ACT) and DMA'd back bf16 with few,
    large descriptors (store issue cost ~0.7us each dominates the tail);
    host reassembles the row interleave in fp32.
"""

import numpy as np
import ml_dtypes

import concourse.bass as bass
import concourse.bacc as bacc
import concourse.tile as tile
import concourse.mybir as mybir
from concourse.bass_utils import run_bass_kernel_spmd

F32 = mybir.dt.float32
BF16 = mybir.dt.bfloat16
MULT = mybir.AluOpType.mult
ADD = mybir.AluOpType.add

B, CIN, COUT, KS = 8, 64, 64, 3
H, W = 256, 256
KNUM = 4
D1, D2 = COUT * KS, CIN * KS          # 192, 192
D2R = D2 // 2 + 1                     # 97
NF = D1 * D2R                         # 18624

NPAIR = 128          # output row pairs (2u+1, 2u+2), u = -1..127
SLOT = W + 2         # 258: [pad, 256 cols, pad] per row-slot
# x chunk slot boundaries: small first chunk so the first conv blocks' slots
# land early (startup is early-phase DMA-bandwidth bound)
XB = [0, 6, 22, 30, 46, 62, 78, 94, 110, 126, 128]
NCH = len(XB) - 1


def _host_constants():
    fh = np.fft.fftfreq(D1)
    fw = np.fft.rfftfreq(D2)
    dist = np.sqrt(fh[:, None] ** 2 + fw[None, :] ** 2)
    idx = np.argsort(dist.ravel(), kind='stable')
    FH = (idx // D2R).astype(np.int64)
    FW = (idx % D2R).astype(np.int64)
    perm = FH * D2R + FW
    inv = np.empty(NF, dtype=np.int64)
    inv[perm] = np.arange(NF)

    hh = np.arange(D1)
    ang = 2.0 * np.pi * np.outer(hh, hh) / D1
    # att scale 2/KNUM = 0.5 folded into the stage-1 basis
    Cb = (np.cos(ang) * (0.5 / D1)).astype(np.float32)
    Sb = (np.sin(ang) * (0.5 / D1)).astype(np.float32)
    C2 = np.concatenate([Cb[:96], Cb[96:]], axis=1)           # [96, 384]
    S2 = np.concatenate([Sb[:96], Sb[96:]], axis=1)           # [96, 384]
    cs = np.concatenate([C2, S2], axis=1).astype(ml_dtypes.bfloat16)

    w_ = np.arange(D2R)
    n_ = np.arange(D2)
    alpha = np.full(D2R, 2.0); alpha[0] = 1.0; alpha[-1] = 1.0
    beta = np.full(D2R, 2.0); beta[0] = 0.0; beta[-1] = 0.0
    ang2 = 2.0 * np.pi * np.outer(w_, n_) / D2
    A = (alpha[:, None] * np.cos(ang2) / D2).astype(np.float32)   # [97, 192]
    Bm = (beta[:, None] * np.sin(ang2) / D2).astype(np.float32)
    ab = np.concatenate(
        [A[:, dx::3] for dx in range(3)] + [-Bm[:, dx::3] for dx in range(3)],
        axis=1,
    ).astype(ml_dtypes.bfloat16)                               # [97, 384]
    return inv, cs, ab


_INV, _CS, _AB = _host_constants()

# (which, dx) order used in the conv weight loop; t_sb index = 2*dx + which
_WSEQ = [(0, 0), (1, 0), (0, 1), (1, 1), (0, 2), (1, 2)]
# valid quadrants (J, r, c0) per `which` (c0 = J - r + 2*which)
_QUADS = {
    0: [(0, 0, 0), (1, 0, 1), (1, 1, 0)],
    1: [(0, 0, 2), (0, 1, 1), (1, 1, 2)],
}
_ZQUAD = {0: (0, 1), 1: (1, 0)}  # zero quadrant (J, r)


def _emit_kernel(tc):
    nc = tc.nc
    from contextlib import ExitStack

    # x: [parity, cin, slot, 258] bf16, host-padded (col 0 and 257 are zeros)
    x_in = nc.dram_tensor("x_in", [2, CIN, NPAIR, SLOT], BF16,
                          kind="ExternalInput").ap()
    lg_in = nc.dram_tensor("lg_in", [1, KNUM], F32, kind="ExternalInput").ap()
    # spectrum, kernel-major: cols k*(4*97) + q*97 + w with quadrant
    # q in {0:(re,h0), 1:(re,h1), 2:(imneg,h0), 3:(imneg,h1)}
    dd_in = nc.dram_tensor("dd_in", [96, KNUM * 4 * D2R], BF16,
                           kind="ExternalInput").ap()
    cs_in = nc.dram_tensor("cs_in", [96, 4 * D1], BF16,
                           kind="ExternalInput").ap()
    ab_in = nc.dram_tensor("ab_in", [D2R, 6 * 64], BF16,
                           kind="ExternalInput").ap()
    # device-side output layout: plane r=0 slot s = row 2s-1, plane r=1 slot
    # s = row 2s. Keeps every store descriptor contiguous per partition;
    # host reassembles rows.
    out = nc.dram_tensor("out", [2, COUT, NPAIR + 1, W], BF16,
                         kind="ExternalOutput").ap()

    KW = 4 * D2R  # 388, one kernel's width in dd

    with ExitStack() as ctx:
        cpool = ctx.enter_context(tc.tile_pool(name="cpool", bufs=1))
        xbpool = ctx.enter_context(tc.tile_pool(name="xbpool", bufs=1))
        spool = ctx.enter_context(tc.tile_pool(name="spool", bufs=3))

        # ---- latency-critical small loads. DMA transfers complete roughly
        # in global issue order at ~135 GB/s early on, so the T-synthesis
        # chain loads must be the first issues on every engine; everything
        # else is gated behind progress markers of the T chain.
        l_sb = cpool.tile([1, KNUM], F32, name="l_sb")
        dd_sb = cpool.tile([96, KNUM * KW], BF16, name="dd_sb")
        cs_sb = cpool.tile([96, 4 * D1], BF16, name="cs_sb")
        nc2_sb = cpool.tile([96, 2 * D1], BF16, name="nc2_sb")
        ab_sb = cpool.tile([D2R, 6 * 64], BF16, name="ab_sb")

        nc.sync.dma_start(out=l_sb[:], in_=lg_in)
        nc.sync.dma_start(out=dd_sb[:, 0:KW], in_=dd_in[:, 0:KW])           # k0
        nc.gpsimd.dma_start(out=dd_sb[:, KW:2 * KW], in_=dd_in[:, KW:2 * KW])
        nc.sync.dma_start(out=dd_sb[:, 2 * KW:3 * KW],
                          in_=dd_in[:, 2 * KW:3 * KW])                      # k2
        nc.gpsimd.dma_start(out=dd_sb[:, 3 * KW:4 * KW],
                            in_=dd_in[:, 3 * KW:4 * KW])                    # k3
        nc.sync.dma_start(out=cs_sb[:, 0:2 * D1], in_=cs_in[:, 0:2 * D1])   # C2
        nc.gpsimd.dma_start(out=cs_sb[:, 2 * D1:4 * D1],
                            in_=cs_in[:, 2 * D1:4 * D1])                    # S2
        # ab is small and gates stage 2; keep it inside the first cohort
        nc.gpsimd.dma_start(out=ab_sb[:], in_=ab_in)

        # sigmoid first in the ACT stream (its act table loads before it)
        sig_sb = cpool.tile([1, KNUM], F32, name="sig_sb")
        sig_i = nc.scalar.activation(sig_sb[:], l_sb[:],
                                     mybir.ActivationFunctionType.Sigmoid)
        ones_sb = cpool.tile([1, 128], F32, name="ones_sb")
        nc.vector.memset(ones_sb[:], 1.0)

        # ---- per-sample spectrum mixing: 4 wide DVE FMAs (fp32 chain, the
        # last op writes bf16). Layout: [re_h0 | re_h1 | imneg_h0 | imneg_h1].
        mm_bf = cpool.tile([96, KW], BF16, name="mm_bf")
        macc = cpool.tile([96, KW], F32, name="macc")

        gtre_sb = cpool.tile([D2R, D1 + 2], BF16, name="gtre_sb")
        gtim_sb = cpool.tile([D2R, D1 + 2], BF16, name="gtim_sb")
        t_sb = [cpool.tile([128, 128], BF16, name=f"t_sb_{i}") for i in range(6)]

        dftps_cm = tc.tile_pool(name="dftps", bufs=1, space="PSUM")
        dpool = dftps_cm.__enter__()
        # att broadcast: [128, 4] = ones.T @ sig
        att_ps = dpool.tile([128, KNUM], F32, name="att_ps")
        nc.tensor.matmul(att_ps[:], ones_sb[:], sig_sb[:], start=True, stop=True)
        att_sb = cpool.tile([96, KNUM], F32, name="att_sb")
        att_cp = nc.vector.tensor_copy(att_sb[:], att_ps[0:96, :])

        nc.vector.tensor_scalar_mul(macc[:], dd_sb[:, 0:KW], att_sb[:, 0:1])
        fma1 = nc.vector.scalar_tensor_tensor(
            macc[:], dd_sb[:, KW:2 * KW], att_sb[:, 1:2], macc[:], MULT, ADD)
        nc.vector.scalar_tensor_tensor(
            macc[:], dd_sb[:, 2 * KW:3 * KW], att_sb[:, 2:3], macc[:],
            MULT, ADD)
        fma3 = nc.vector.scalar_tensor_tensor(
            mm_bf[:], dd_sb[:, 3 * KW:4 * KW],
            att_sb[:, 3:4], macc[:], MULT, ADD)

        # ---- x chunk loads: slot t holds rows (2t, 2t+1). Issue cohorts:
        # chunk0 slots 0-7 behind att_cp, chunk0 tail + chunk1 first half
        # behind the k1 mixing FMA, the rest behind the full dft chain.
        xch = []
        xdmas = []
        gated = {id(att_cp): [], id(fma1): [], id(fma3): []}
        for c in range(NCH):
            # chunks overlap one slot so 2-pair windows never cross a tile
            # boundary; the last chunk ends at slot 127
            nsl = XB[c + 1] - XB[c] + (1 if c + 1 < NCH else 0)
            xb = xbpool.tile([128, nsl * SLOT], BF16, name=f"xb{c}")
            if c == 0:
                engs = [nc.scalar, nc.sync, nc.scalar, nc.gpsimd]
                ranges = ((0, nsl),)
                gates = (fma1,)
            elif c == 1:
                engs = [nc.sync, nc.gpsimd, nc.scalar, nc.scalar,
                        nc.gpsimd, nc.sync, nc.gpsimd, nc.sync]
                ranges = ((0, 9), (9, nsl))
                gates = ('tcopy', 'tcopy')
            else:
                engs = None
            if c < 2:
                ei = 0
                for (s0, s1), gate in zip(ranges, gates):
                    for par in range(2):
                        for p0, p1 in ((0, 32), (32, 64)):
                            di = engs[ei].dma_start(
                                out=xb[64 * par + p0: 64 * par + p1,
                                       s0 * SLOT:s1 * SLOT],
                                in_=x_in[par, p0:p1,
                                         XB[c] + s0:XB[c] + s1, :])
                            if engs[ei] is nc.scalar:
                                # keep the ACT stream clear for the sigmoid
                                bass._add_dep_helper(
                                    di.ins, sig_i.ins,
                                    reason="sigmoid before scalar descgen")
                            if gate == 'tcopy':
                                xdmas.append(di)
                            else:
                                gated[id(gate)].append(di)
                            ei += 1
            else:
                # alternate gpsimd/sync so neither queue falls behind the
                # conv's slot consumption mid-stream
                for par in range(2):
                    for pi, (p0, p1) in enumerate(((0, 32), (32, 64))):
                        eng = nc.gpsimd if pi == 0 else nc.sync
                        di = eng.dma_start(
                            out=xb[64 * par + p0: 64 * par + p1, 0:nsl * SLOT],
                            in_=x_in[par, p0:p1, XB[c]:XB[c] + nsl, :])
                        xdmas.append(di)
            xch.append(xb)
        for di in gated[id(att_cp)]:
            bass._add_dep_helper(di.ins, att_cp.ins,
                                 reason="bulk x yields HBM to T-chain loads")
        for di in gated[id(fma1)]:
            bass._add_dep_helper(di.ins, fma1.ins,
                                 reason="bulk x yields HBM to T-chain loads")
        for di in gated[id(fma3)]:
            bass._add_dep_helper(di.ins, fma3.ins,
                                 reason="bulk x yields HBM to T-chain loads")

        # -C2 basis computed on ACT (saves 74KB of critical-path loads);
        # emitted after scalar's gated x pieces so it doesn't block them
        nc.scalar.mul(nc2_sb[:], cs_sb[:, 0:2 * D1], -1.0)

        def slot_rhs(s, dx, npair=1):
            # [128, npair, W] window starting at slot s (npair<=2; both slots
            # live in one chunk thanks to the 1-slot overlap)
            for c in range(NCH):
                if s < XB[c + 1]:
                    break
            loc = s - XB[c]
            if npair == 1:
                return xch[c][:, loc * SLOT + dx: loc * SLOT + dx + W]
            v = xch[c].rearrange("p (t s) -> p t s", s=SLOT)
            return v[:, loc:loc + npair, dx:dx + W]

        if True:
            def mview(q):
                return mm_bf[:, q * D2R:(q + 1) * D2R]

            def csv(which, h):
                if which == 2:
                    return nc2_sb[:, h * D1:(h + 1) * D1]
                return cs_sb[:, which * 2 * D1 + h * D1:
                             which * 2 * D1 + (h + 1) * D1]

            # ---- stage 1: GT = M^T . [C|S|-C], contraction over h halves
            gtre_ps = dpool.tile([D2R, D1], F32, name="gtre_ps")
            gtim_ps = dpool.tile([D2R, D1], F32, name="gtim_ps")
            # gtre = mre.C + mineg.S ; gtim = mre.S + mineg.(-C)
            nc.tensor.matmul(gtre_ps[:], mview(0), csv(0, 0),
                             start=True, stop=False)
            nc.tensor.matmul(gtim_ps[:], mview(0), csv(1, 0),
                             start=True, stop=False)
            nc.tensor.matmul(gtre_ps[:], mview(1), csv(0, 1),
                             start=False, stop=False)
            nc.tensor.matmul(gtim_ps[:], mview(1), csv(1, 1),
                             start=False, stop=False)
            nc.tensor.matmul(gtre_ps[:], mview(2), csv(1, 0),
                             start=False, stop=False)
            nc.tensor.matmul(gtim_ps[:], mview(2), csv(2, 0),
                             start=False, stop=False)
            nc.tensor.matmul(gtre_ps[:], mview(3), csv(1, 1),
                             start=False, stop=True)
            nc.tensor.matmul(gtim_ps[:], mview(3), csv(2, 1),
                             start=False, stop=True)
            nc.vector.tensor_copy(gtre_sb[:, 0:D1], gtre_ps[:])
            nc.vector.tensor_copy(gtim_sb[:, 0:D1], gtim_ps[:])

            # ---- stage 2: six conv weight matrices T[(ci,j),(co,r)]
            def gview(g, c0):
                return g[:, c0:c0 + D1].rearrange(
                    "w (co th) -> w co th", th=3)[:, :, 0:1]

            t_copy = None
            for i, (which, dx) in enumerate(_WSEQ):
                t_ps = dpool.tile([128, 128], F32, name="t_ps", bufs=3)
                zj, zr = _ZQUAD[which]
                nc.vector.memset(t_ps[64 * zj:64 * zj + 64, 64 * zr:64 * zr + 64], 0.0)
                for (J, r, c0) in _QUADS[which]:
                    o = t_ps[64 * J:64 * J + 64, 64 * r:64 * r + 64]
                    nc.tensor.matmul(o, ab_sb[:, dx * 64:(dx + 1) * 64],
                                     gview(gtre_sb, c0), start=True, stop=False)
                    nc.tensor.matmul(o, ab_sb[:, (3 + dx) * 64:(4 + dx) * 64],
                                     gview(gtim_sb, c0), start=False, stop=True)
                t_copy = nc.vector.tensor_copy(t_sb[2 * dx + which][:], t_ps[:])
        dftps_cm.__exit__(None, None, None)
        # late x chunks wait for the dft chain to clear the HBM/SDMA path
        for di in xdmas:
            bass._add_dep_helper(di.ins, t_copy.ins,
                                 reason="late x chunks yield HBM to dft chain")

        # ---- conv over row pairs
        # staging groups over pair slots s = u+1 in [0, 129): big early, small
        # at the end so the last stores drain quickly
        gsizes = [16] * 6 + [8, 8, 8, 4, 2, 1, 1, 1]
        gstart = np.cumsum([0] + gsizes).tolist()

        def group_of(s):
            for gi in range(len(gsizes)):
                if s < gstart[gi + 1]:
                    return gi, s - gstart[gi]
            raise AssertionError

        # units: (-1,) special, (0,1), (2,3), ..., (124,125), (126,), (127,)
        units = [(-1,)] + [(u, u + 1) for u in range(0, 126, 2)] + [(126,), (127,)]

        with tc.tile_pool(name="convps", bufs=1, space="PSUM") as cps:
            # 8 preallocated PSUM bank tiles reused round-robin: identical
            # schedule to per-block allocation with less pool bookkeeping.
            ptiles = [cps.tile([128, 2 * W], F32, name=f"pp{i}")
                      for i in range(8)]
            pidx = [0]
            staging = {}

            def get_staging(gi):
                if gi not in staging:
                    if gi >= 9:
                        # small late groups get dedicated slots so the final
                        # copies never wait on store completions
                        staging[gi] = spool.tile(
                            [128, gsizes[gi] * W], BF16,
                            name=f"staging_l{gi}", bufs=1)
                    else:
                        staging[gi] = spool.tile(
                            [128, gsizes[gi] * W], BF16,
                            name=f"staging{gsizes[gi]}")
                return staging[gi]

            def unit_mms(un):
                L = []
                for wh, dx in _WSEQ:
                    if wh == 0 and un[0] < 0:
                        continue
                    if wh == 1 and un[0] > 126:
                        continue
                    L.append((wh, dx))
                return L

            def emit_block(uns):
                tiles = {}
                for un in uns:
                    tiles[un] = ptiles[pidx[0] % 8][:, 0:len(un) * W]
                    pidx[0] += 1
                plan = {un: unit_mms(un) for un in uns}
                for k, (wh, dx) in enumerate(_WSEQ):
                    for un in uns:
                        if (wh, dx) not in plan[un]:
                            continue
                        i = plan[un].index((wh, dx))
                        rhs = slot_rhs(un[0] + (0 if wh == 0 else 1), dx,
                                       len(un))
                        nc.tensor.matmul(
                            tiles[un][:], t_sb[2 * dx + wh][:], rhs,
                            start=(i == 0), stop=(i == len(plan[un]) - 1),
                            skip_group_check=True)
                for un in uns:
                    for j, u in enumerate(un):
                        gi, si = group_of(u + 1)
                        st = get_staging(gi)[:, si * W:(si + 1) * W]
                        src = tiles[un][:, j * W:(j + 1) * W]
                        if u == -1:
                            nc.scalar.copy(st[64:128, :], src[64:128, :])
                        elif u == 127:
                            nc.scalar.copy(st[0:64, :], src[0:64, :])
                        elif (j == 0 and len(un) == 2 and
                              group_of(un[1] + 1)[0] == gi):
                            # both halves land in the same staging tile: one
                            # wide copy, alternating engines per unit
                            st2 = get_staging(gi)[:, si * W:(si + 2) * W]
                            if (u // 2) % 2 == 0:
                                nc.vector.tensor_copy(st2, tiles[un][:])
                            else:
                                nc.scalar.copy(st2, tiles[un][:])
                            break
                        elif u % 2 == 0:
                            nc.vector.tensor_copy(st, src)
                        else:
                            nc.scalar.copy(st, src)
                    for u in un:
                        gi, si = group_of(u + 1)
                        if si == gsizes[gi] - 1:
                            emit_stores(gi)

            store_cnt = [0]

            def store_dma(dst, src, late=False):
                # round-robin store engines; sync carries half the x loads,
                # so early stores go on scalar/gpsimd
                if late == 0:
                    engs = [nc.scalar, nc.gpsimd]
                elif late == 1:
                    engs = [nc.scalar, nc.sync, nc.gpsimd]
                else:
                    engs = [nc.sync, nc.scalar, nc.gpsimd]
                eng = engs[store_cnt[0] % len(engs)]
                store_cnt[0] += 1
                eng.dma_start(out=dst, in_=src)

            def emit_stores(gi):
                stg = staging.pop(gi)
                s0, s1 = gstart[gi], gstart[gi + 1]
                late = 0 if gi < 4 else (1 if gi < 9 else 2)
                sv = stg.rearrange("p (g w) -> p g w", w=W)
                if gi == 0:
                    # row 0 from pair u=-1 -> plane 1, slot 0
                    store_dma(out[1, :, 0:1, :], sv[64:128, 0:1, :])
                # full pairs in this group: slots max(s0,1) .. min(s1,128)-1
                fa, fb = max(s0, 1), min(s1, 128)
                # small transfers near the end so the post-conv drain is short
                run = 8 if gi < 4 else 4
                va = fa
                while va < fb:
                    vb = min(fb, va + run)
                    G = vb - va
                    store_dma(out[0, :, va:vb, :],
                              sv[0:64, va - s0:va - s0 + G, :], late)
                    store_dma(out[1, :, va:vb, :],
                              sv[64:128, va - s0:va - s0 + G, :], late)
                    va = vb
                if s1 == 129:
                    # row 255 from pair u=127 -> plane 0, slot 128
                    store_dma(out[0, :, NPAIR:NPAIR + 1, :],
                              sv[0:64, 128 - s0:129 - s0, :], True)

            # blocks of up to 4 units
            ui = 0
            while ui < len(units):
                emit_block(units[ui:ui + 4])
                ui += 4


_NC_CACHE = None


def _build_nc():
    global _NC_CACHE
    if _NC_CACHE is None:
        nc = bacc.Bacc("TRN2", target_bir_lowering=False, debug=False,
                       num_devices=B)
        with tile.TileContext(nc) as tc:
            _emit_kernel(tc)
        nc.compile()
        _NC_CACHE = nc
    return _NC_CACHE


def _in_maps(x, k_att_logits, dft_weight):
    x = np.asarray(x, dtype=np.float32)
    lg = np.asarray(k_att_logits, dtype=np.float32)
    dw = np.asarray(dft_weight, dtype=np.float32)

    # x -> bf16, parity-split rows, host-inserted zero pad columns
    xp = np.zeros((B, 2, CIN, NPAIR, SLOT), dtype=ml_dtypes.bfloat16)
    xv = x.reshape(B, CIN, NPAIR, 2, W).transpose(0, 3, 1, 2, 4)  # [b,j,c,t,w]
    xp[:, :, :, :, 1:1 + W] = xv.astype(ml_dtypes.bfloat16)

    # host-side gather: dense half-spectrum layout, kernel-major quadrants
    dftP = dw[:, _INV, :].reshape(KNUM, 2, 96, D2R, 2)   # [k, half, p, w, c]
    re = dftP[..., 0]             # [k, half, p, w]
    imneg = -dftP[..., 1]
    quad = np.stack([re[:, 0], re[:, 1], imneg[:, 0], imneg[:, 1]],
                    axis=1)       # [k, q, p, w]
    dd = np.ascontiguousarray(
        quad.transpose(2, 0, 1, 3).reshape(96, KNUM * 4 * D2R)
    ).astype(ml_dtypes.bfloat16)

    maps = []
    for b in range(B):
        maps.append({
            "x_in": np.ascontiguousarray(xp[b]),
            "lg_in": lg[b:b + 1],
            "dd_in": dd,
            "cs_in": _CS,
            "ab_in": _AB,
        })
    return maps


def _execute(x, k_att_logits, dft_weight, trace=False, **trace_kwargs):
    nc = _build_nc()
    res = run_bass_kernel_spmd(
        nc, _in_maps(x, k_att_logits, dft_weight),
        core_ids=list(range(B)), trace=trace, **trace_kwargs)
    out = np.empty((B, COUT, H, W), dtype=np.float32)
    for b in range(B):
        dev = res.results[b]["out"]      # [2, COUT, NPAIR+1, W] bf16
        out[b, :, 1::2, :] = dev[0, :, 1:NPAIR + 1, :].astype(np.float32)
        out[b, :, 0::2, :] = dev[1, :, 0:NPAIR, :].astype(np.float32)
    return out, res


def kernel(x, k_att_logits, dft_weight):
    out, _ = _execute(x, k_att_logits, dft_weight)
    return out.astype(np.float32)


# revision 23
# speedup vs baseline: 1.0296x; 1.0125x over previous
"""Trainium2 Bass kernel for nn_FDConv (per-sample frequency-domain-synthesized
3x3 grouped conv).

Strategy (data-parallel over batch, 1 sample per NeuronCore):
  - host: permute dft_weight into dense half-spectrum layout (pure gather),
    precompute DFT basis matrices (incl. a negated copy of C so no on-device
    negations are needed), stage x as bf16 in a padded parity-split layout.
  - device per core:
      att = sigmoid(logits), broadcast via K=1 matmul
      M   = per-sample mixed spectrum: 4 wide DVE FMA ops over the 4 kernels
            (kernel-major layout -> one [96, 388] op per kernel, final op
            fuses the bf16 cast)
      GT  = M^T . [C | S | -C]  (stage-1 iFFT along axis 0, 8 matmuls)
      T[which,dx] = A_dx^T . GT_re - B_dx^T . GT_im  (stage-2 irfft + reshape
            to six 128x128 conv weight mats)
      conv: x bf16 with even rows on partitions 0-63 and odd rows on 64-127;
            two output row pairs per matmul group: six K=128/M=128/N=512
            matmuls (75% PE utilization bound is structural for this shape)
            accumulated in 8 preallocated PSUM banks.
  - outputs copied PSUM->SBUF as bf16 (DVE/ACT) and DMA'd back bf16 with few,
    large descriptors (store issue cost ~0.7us each dominates the tail);
    host reassembles the row interleave in fp32.
"""

import numpy as np
import ml_dtypes

import concourse.bass as bass
import concourse.bacc as bacc
import concourse.tile as tile
import concourse.mybir as mybir
from concourse.bass_utils import run_bass_kernel_spmd

F32 = mybir.dt.float32
BF16 = mybir.dt.bfloat16
MULT = mybir.AluOpType.mult
ADD = mybir.AluOpType.add

B, CIN, COUT, KS = 8, 64, 64, 3
H, W = 256, 256
KNUM = 4
D1, D2 = COUT * KS, CIN * KS          # 192, 192
D2R = D2 // 2 + 1                     # 97
NF = D1 * D2R                         # 18624

NPAIR = 128          # output row pairs (2u+1, 2u+2), u = -1..127
SLOT = W + 2         # 258: [pad, 256 cols, pad] per row-slot
# x chunk slot boundaries: small first chunk so the first conv blocks' slots
# land early (startup is early-phase DMA-bandwidth bound)
XB = [0, 6, 22, 30, 46, 62, 78, 94, 110, 126, 128]
NCH = len(XB) - 1


def _host_constants():
    fh = np.fft.fftfreq(D1)
    fw = np.fft.rfftfreq(D2)
    dist = np.sqrt(fh[:, None] ** 2 + fw[None, :] ** 2)
    idx = np.argsort(dist.ravel(), kind='stable')
    FH = (idx // D2R).astype(np.int64)
    FW = (idx % D2R).astype(np.int64)
    perm = FH * D2R + FW
    inv = np.empty(NF, dtype=np.int64)
    inv[perm] = np.arange(NF)

    hh = np.arange(D1)
    ang = 2.0 * np.pi * np.outer(hh, hh) / D1
    # att scale 2/KNUM = 0.5 folded into the stage-1 basis
    Cb = (np.cos(ang) * (0.5 / D1)).astype(np.float32)
    Sb = (np.sin(ang) * (0.5 / D1)).astype(np.float32)
    C2 = np.concatenate([Cb[:96], Cb[96:]], axis=1)           # [96, 384]
    S2 = np.concatenate([Sb[:96], Sb[96:]], axis=1)           # [96, 384]
    cs = np.concatenate([C2, S2], axis=1).astype(ml_dtypes.bfloat16)

    w_ = np.arange(D2R)
    n_ = np.arange(D2)
    alpha = np.full(D2R, 2.0); alpha[0] = 1.0; alpha[-1] = 1.0
    beta = np.full(D2R, 2.0); beta[0] = 0.0; beta[-1] = 0.0
    ang2 = 2.0 * np.pi * np.outer(w_, n_) / D2
    A = (alpha[:, None] * np.cos(ang2) / D2).astype(np.float32)   # [97, 192]
    Bm = (beta[:, None] * np.sin(ang2) / D2).astype(np.float32)
    ab = np.concatenate(
        [A[:, dx::3] for dx in range(3)] + [-Bm[:, dx::3] for dx in range(3)],
        axis=1,
    ).astype(ml_dtypes.bfloat16)                               # [97, 384]
    return inv, cs, ab


_INV, _CS, _AB = _host_constants()

# (which, dx) order used in the conv weight loop; t_sb index = 2*dx + which
_WSEQ = [(0, 0), (1, 0), (0, 1), (1, 1), (0, 2), (1, 2)]
# valid quadrants (J, r, c0) per `which` (c0 = J - r + 2*which)
_QUADS = {
    0: [(0, 0, 0), (1, 0, 1), (1, 1, 0)],
    1: [(0, 0, 2), (0, 1, 1), (1, 1, 2)],
}
_ZQUAD = {0: (0, 1), 1: (1, 0)}  # zero quadrant (J, r)


def _emit_kernel(tc):
    nc = tc.nc
    from contextlib import ExitStack

    # x: [parity, cin, slot, 258] bf16, host-padded (col 0 and 257 are zeros)
    x_in = nc.dram_tensor("x_in", [2, CIN, NPAIR, SLOT], BF16,
                          kind="ExternalInput").ap()
    lg_in = nc.dram_tensor("lg_in", [1, KNUM], F32, kind="ExternalInput").ap()
    # spectrum, kernel-major: cols k*(4*97) + q*97 + w with quadrant
    # q in {0:(re,h0), 1:(re,h1), 2:(imneg,h0), 3:(imneg,h1)}
    dd_in = nc.dram_tensor("dd_in", [96, KNUM * 4 * D2R], BF16,
                           kind="ExternalInput").ap()
    cs_in = nc.dram_tensor("cs_in", [96, 4 * D1], BF16,
                           kind="ExternalInput").ap()
    ab_in = nc.dram_tensor("ab_in", [D2R, 6 * 64], BF16,
                           kind="ExternalInput").ap()
    # device-side output layout: plane r=0 slot s = row 2s-1, plane r=1 slot
    # s = row 2s. Keeps every store descriptor contiguous per partition;
    # host reassembles rows.
    out = nc.dram_tensor("out", [2, COUT, NPAIR + 1, W], BF16,
                         kind="ExternalOutput").ap()

    KW = 4 * D2R  # 388, one kernel's width in dd

    with ExitStack() as ctx:
        cpool = ctx.enter_context(tc.tile_pool(name="cpool", bufs=1))
        xbpool = ctx.enter_context(tc.tile_pool(name="xbpool", bufs=1))
        spool = ctx.enter_context(tc.tile_pool(name="spool", bufs=3))

        # ---- latency-critical small loads. DMA transfers complete roughly
        # in global issue order at ~135 GB/s early on, so the T-synthesis
        # chain loads must be the first issues on every engine; everything
        # else is gated behind progress markers of the T chain.
        l_sb = cpool.tile([1, KNUM], F32, name="l_sb")
        dd_sb = cpool.tile([96, KNUM * KW], BF16, name="dd_sb")
        cs_sb = cpool.tile([96, 4 * D1], BF16, name="cs_sb")
        nc2_sb = cpool.tile([96, 2 * D1], BF16, name="nc2_sb")
        ab_sb = cpool.tile([D2R, 6 * 64], BF16, name="ab_sb")

        nc.sync.dma_start(out=l_sb[:], in_=lg_in)
        nc.sync.dma_start(out=dd_sb[:, 0:KW], in_=dd_in[:, 0:KW])           # k0
        nc.gpsimd.dma_start(out=dd_sb[:, KW:2 * KW], in_=dd_in[:, KW:2 * KW])
        nc.sync.dma_start(out=dd_sb[:, 2 * KW:3 * KW],
                          in_=dd_in[:, 2 * KW:3 * KW])                      # k2
        nc.gpsimd.dma_start(out=dd_sb[:, 3 * KW:4 * KW],
                            in_=dd_in[:, 3 * KW:4 * KW])                    # k3
        nc.sync.dma_start(out=cs_sb[:, 0:2 * D1], in_=cs_in[:, 0:2 * D1])   # C2
        nc.gpsimd.dma_start(out=cs_sb[:, 2 * D1:4 * D1],
                            in_=cs_in[:, 2 * D1:4 * D1])                    # S2
        # ab is small and gates stage 2; keep it inside the first cohort
        nc.gpsimd.dma_start(out=ab_sb[:], in_=ab_in)

        # sigmoid first in the ACT stream (its act table loads before it)
        sig_sb = cpool.tile([1, KNUM], F32, name="sig_sb")
        sig_i = nc.scalar.activation(sig_sb[:], l_sb[:],
                                     mybir.ActivationFunctionType.Sigmoid)
        ones_sb = cpool.tile([1, 128], F32, name="ones_sb")
        nc.vector.memset(ones_sb[:], 1.0)

        # ---- per-sample spectrum mixing: 4 wide DVE FMAs (fp32 chain, the
        # last op writes bf16). Layout: [re_h0 | re_h1 | imneg_h0 | imneg_h1].
        mm_bf = cpool.tile([96, KW], BF16, name="mm_bf")
        macc = cpool.tile([96, KW], F32, name="macc")

        gtre_sb = cpool.tile([D2R, D1 + 2], BF16, name="gtre_sb")
        gtim_sb = cpool.tile([D2R, D1 + 2], BF16, name="gtim_sb")
        t_sb = [cpool.tile([128, 128], BF16, name=f"t_sb_{i}") for i in range(6)]

        dftps_cm = tc.tile_pool(name="dftps", bufs=1, space="PSUM")
        dpool = dftps_cm.__enter__()
        # att broadcast: [128, 4] = ones.T @ sig
        att_ps = dpool.tile([128, KNUM], F32, name="att_ps")
        nc.tensor.matmul(att_ps[:], ones_sb[:], sig_sb[:], start=True, stop=True)
        att_sb = cpool.tile([96, KNUM], F32, name="att_sb")
        att_cp = nc.vector.tensor_copy(att_sb[:], att_ps[0:96, :])

        nc.vector.tensor_scalar_mul(macc[:], dd_sb[:, 0:KW], att_sb[:, 0:1])
        fma1 = nc.vector.scalar_tensor_tensor(
            macc[:], dd_sb[:, KW:2 * KW], att_sb[:, 1:2], macc[:], MULT, ADD)
        nc.vector.scalar_tensor_tensor(
            macc[:], dd_sb[:, 2 * KW:3 * KW], att_sb[:, 2:3], macc[:],
            MULT, ADD)
        fma3 = nc.vector.scalar_tensor_tensor(
            mm_bf[:], dd_sb[:, 3 * KW:4 * KW],
            att_sb[:, 3:4], macc[:], MULT, ADD)

        # ---- x chunk loads: slot t holds rows (2t, 2t+1). Issue cohorts:
        # chunk0 slots 0-7 behind att_cp, chunk0 tail + chunk1 first half
        # behind the k1 mixing FMA, the rest behind the full dft chain.
        xch = []
        xdmas = []
        gated = {id(att_cp): [], id(fma1): [], id(fma3): []}
        for c in range(NCH):
            # chunks overlap one slot so 2-pair windows never cross a tile
            # boundary; the last chunk ends at slot 127
            nsl = XB[c + 1] - XB[c] + (1 if c + 1 < NCH else 0)
            xb = xbpool.tile([128, nsl * SLOT], BF16, name=f"xb{c}")
            if c == 0:
                engs = [nc.scalar, nc.sync, nc.scalar, nc.gpsimd]
                ranges = ((0, nsl),)
                gates = (fma1,)
            elif c == 1:
                engs = [nc.sync, nc.gpsimd, nc.scalar, nc.scalar,
                        nc.gpsimd, nc.sync, nc.gpsimd, nc.sync]
                ranges = ((0, 9), (9, nsl))
                gates = ('tcopy', 'tcopy')
            else:
                engs = None
            if c < 2:
                ei = 0
                for (s0, s1), gate in zip(ranges, gates):
                    for par in range(2):
                        for p0, p1 in ((0, 32), (32, 64)):
                            di = engs[ei].dma_start(
                                out=xb[64 * par + p0: 64 * par + p1,
                                       s0 * SLOT:s1 * SLOT],
                                in_=x_in[par, p0:p1,
                                         XB[c] + s0:XB[c] + s1, :])
                            if engs[ei] is nc.scalar:
                                # keep the ACT stream clear for the sigmoid
                                bass._add_dep_helper(
                                    di.ins, sig_i.ins,
                                    reason="sigmoid before scalar descgen")
                            if gate == 'tcopy':
                                xdmas.append(di)
                            else:
                                gated[id(gate)].append(di)
                            ei += 1
            else:
                # alternate gpsimd/sync so neither queue falls behind the
                # conv's slot consumption mid-stream
                for par in range(2):
                    for pi, (p0, p1) in enumerate(((0, 32), (32, 64))):
                        eng = nc.gpsimd if pi == 0 else nc.sync
                        di = eng.dma_start(
                            out=xb[64 * par + p0: 64 * par + p1, 0:nsl * SLOT],
                            in_=x_in[par, p0:p1, XB[c]:XB[c] + nsl, :])
                        xdmas.append(di)
            xch.append(xb)
        for di in gated[id(att_cp)]:
            bass._add_dep_helper(di.ins, att_cp.ins,
                                 reason="bulk x yields HBM to T-chain loads")
        for di in gated[id(fma1)]:
            bass._add_dep_helper(di.ins, fma1.ins,
                                 reason="bulk x yields HBM to T-chain loads")
        for di in gated[id(fma3)]:
            bass._add_dep_helper(di.ins, fma3.ins,
                                 reason="bulk x yields HBM to T-chain loads")

        # -C2 basis computed on ACT (saves 74KB of critical-path loads);
        # emitted after scalar's gated x pieces so it doesn't block them
        nc.scalar.mul(nc2_sb[:], cs_sb[:, 0:2 * D1], -1.0)

        def slot_rhs(s, dx, npair=1):
            # [128, npair, W] window starting at slot s (npair<=2; both slots
            # live in one chunk thanks to the 1-slot overlap)
            for c in range(NCH):
                if s < XB[c + 1]:
                    break
            loc = s - XB[c]
            if npair == 1:
                return xch[c][:, loc * SLOT + dx: loc * SLOT + dx + W]
            v = xch[c].rearrange("p (t s) -> p t s", s=SLOT)
            return v[:, loc:loc + npair, dx:dx + W]

        if True:
            def mview(q):
                return mm_bf[:, q * D2R:(q + 1) * D2R]

            def csv(which, h):
                if which == 2:
                    return nc2_sb[:, h * D1:(h + 1) * D1]
                return cs_sb[:, which * 2 * D1 + h * D1:
                             which * 2 * D1 + (h + 1) * D1]

            # ---- stage 1: GT = M^T . [C|S|-C], contraction over h halves
            gtre_ps = dpool.tile([D2R, D1], F32, name="gtre_ps")
            gtim_ps = dpool.tile([D2R, D1], F32, name="gtim_ps")
            # gtre = mre.C + mineg.S ; gtim = mre.S + mineg.(-C)
            nc.tensor.matmul(gtre_ps[:], mview(0), csv(0, 0),
                             start=True, stop=False)
            nc.tensor.matmul(gtim_ps[:], mview(0), csv(1, 0),
                             start=True, stop=False)
            nc.tensor.matmul(gtre_ps[:], mview(1), csv(0, 1),
                             start=False, stop=False)
            nc.tensor.matmul(gtim_ps[:], mview(1), csv(1, 1),
                             start=False, stop=False)
            nc.tensor.matmul(gtre_ps[:], mview(2), csv(1, 0),
                             start=False, stop=False)
            nc.tensor.matmul(gtim_ps[:], mview(2), csv(2, 0),
                             start=False, stop=False)
            nc.tensor.matmul(gtre_ps[:], mview(3), csv(1, 1),
                             start=False, stop=True)
            nc.tensor.matmul(gtim_ps[:], mview(3), csv(2, 1),
                             start=False, stop=True)
            nc.vector.tensor_copy(gtre_sb[:, 0:D1], gtre_ps[:])
            nc.vector.tensor_copy(gtim_sb[:, 0:D1], gtim_ps[:])

            # ---- stage 2: six conv weight matrices T[(ci,j),(co,r)]
            def gview(g, c0):
                return g[:, c0:c0 + D1].rearrange(
                    "w (co th) -> w co th", th=3)[:, :, 0:1]

            t_copy = None
            for i, (which, dx) in enumerate(_WSEQ):
                t_ps = dpool.tile([128, 128], F32, name="t_ps", bufs=3)
                zj, zr = _ZQUAD[which]
                nc.vector.memset(t_ps[64 * zj:64 * zj + 64, 64 * zr:64 * zr + 64], 0.0)
                for (J, r, c0) in _QUADS[which]:
                    o = t_ps[64 * J:64 * J + 64, 64 * r:64 * r + 64]
                    nc.tensor.matmul(o, ab_sb[:, dx * 64:(dx + 1) * 64],
                                     gview(gtre_sb, c0), start=True, stop=False)
                    nc.tensor.matmul(o, ab_sb[:, (3 + dx) * 64:(4 + dx) * 64],
                                     gview(gtim_sb, c0), start=False, stop=True)
                t_copy = nc.vector.tensor_copy(t_sb[2 * dx + which][:], t_ps[:])
        dftps_cm.__exit__(None, None, None)
        # late x chunks wait for the dft chain to clear the HBM/SDMA path
        for di in xdmas:
            bass._add_dep_helper(di.ins, t_copy.ins,
                                 reason="late x chunks yield HBM to dft chain")

        # ---- conv over row pairs
        # staging groups over pair slots s = u+1 in [0, 129): big early, small
        # at the end so the last stores drain quickly
        gsizes = [16] * 6 + [8, 8, 8, 4, 2, 1, 1, 1]
        gstart = np.cumsum([0] + gsizes).tolist()

        def group_of(s):
            for gi in range(len(gsizes)):
                if s < gstart[gi + 1]:
                    return gi, s - gstart[gi]
            raise AssertionError

        # units: (-1,) special, (0,1), (2,3), ..., (124,125), (126,), (127,)
        units = [(-1,)] + [(u, u + 1) for u in range(0, 126, 2)] + [(126,), (127,)]

        with tc.tile_pool(name="convps", bufs=1, space="PSUM") as cps:
            # 8 preallocated PSUM bank tiles reused round-robin: identical
            # schedule to per-block allocation with less pool bookkeeping.
            ptiles = [cps.tile([128, 2 * W], F32, name=f"pp{i}")
                      for i in range(8)]
            pidx = [0]
            staging = {}

            def get_staging(gi):
                if gi not in staging:
                    if gi >= 9:
                        # small late groups get dedicated slots so the final
                        # copies never wait on store completions
                        staging[gi] = spool.tile(
                            [128, gsizes[gi] * W], BF16,
                            name=f"staging_l{gi}", bufs=1)
                    else:
                        staging[gi] = spool.tile(
                            [128, gsizes[gi] * W], BF16,
                            name=f"staging{gsizes[gi]}")
                return staging[gi]

            def unit_mms(un):
                L = []
                for wh, dx in _WSEQ:
                    if wh == 0 and un[0] < 0:
                        continue
                    if wh == 1 and un[0] > 126:
                        continue
                    L.append((wh, dx))
                return L

            def emit_block(uns):
                tiles = {}
                for un in uns:
                    tiles[un] = ptiles[pidx[0] % 8][:, 0:len(un) * W]
                    pidx[0] += 1
                plan = {un: unit_mms(un) for un in uns}
                for k, (wh, dx) in enumerate(_WSEQ):
                    for un in uns:
                        if (wh, dx) not in plan[un]:
                            continue
                        i = plan[un].index((wh, dx))
                        rhs = slot_rhs(un[0] + (0 if wh == 0 else 1), dx,
                                       len(un))
                        nc.tensor.matmul(
                            tiles[un][:], t_sb[2 * dx + wh][:], rhs,
                            start=(i == 0), stop=(i == len(plan[un]) - 1),
                            skip_group_check=True)
                for un in uns:
                    for j, u in enumerate(un):
                        gi, si = group_of(u + 1)
                        st = get_staging(gi)[:, si * W:(si + 1) * W]
                        src = tiles[un][:, j * W:(j + 1) * W]
                        if u == -1:
                            nc.scalar.copy(st[64:128, :], src[64:128, :])
                        elif u == 127:
                            nc.scalar.copy(st[0:64, :], src[0:64, :])
                        elif (j == 0 and len(un) == 2 and
                              group_of(un[1] + 1)[0] == gi):
                            # both halves land in the same staging tile: one
                            # wide copy, alternating engines per unit
                            st2 = get_staging(gi)[:, si * W:(si + 2) * W]
                            if (u // 2) % 2 == 0:
                                nc.vector.tensor_copy(st2, tiles[un][:])
                            else:
                                nc.scalar.copy(st2, tiles[un][:])
                            break
                        elif u % 2 == 0:
                            nc.vector.tensor_copy(st, src)
                        else:
                            nc.scalar.copy(st, src)
                    for u in un:
                        gi, si = group_of(u + 1)
                        if si == gsizes[gi] - 1:
                            emit_stores(gi)

            store_cnt = [0]

            def store_dma(dst, src, late=False):
                # round-robin store engines; sync carries half the x loads,
                # so early stores go on scalar/gpsimd
                if late == 0:
                    engs = [nc.scalar, nc.gpsimd]
                elif late == 1:
                    engs = [nc.scalar, nc.sync, nc.gpsimd]
                else:
                    # keep the final transfers off gpsimd: its SWDGE queue
                    # drains last and would extend the post-conv tail
                    engs = [nc.sync, nc.scalar]
                eng = engs[store_cnt[0] % len(engs)]
                store_cnt[0] += 1
                eng.dma_start(out=dst, in_=src)

            def emit_stores(gi):
                stg = staging.pop(gi)
                s0, s1 = gstart[gi], gstart[gi + 1]
                late = 0 if gi < 4 else (1 if gi < 9 else 2)
                sv = stg.rearrange("p (g w) -> p g w", w=W)
                if gi == 0:
                    # row 0 from pair u=-1 -> plane 1, slot 0
                    store_dma(out[1, :, 0:1, :], sv[64:128, 0:1, :])
                # full pairs in this group: slots max(s0,1) .. min(s1,128)-1
                fa, fb = max(s0, 1), min(s1, 128)
                # small transfers near the end so the post-conv drain is short
                run = 8 if gi < 4 else 4
                va = fa
                while va < fb:
                    vb = min(fb, va + run)
                    G = vb - va
                    store_dma(out[0, :, va:vb, :],
                              sv[0:64, va - s0:va - s0 + G, :], late)
                    store_dma(out[1, :, va:vb, :],
                              sv[64:128, va - s0:va - s0 + G, :], late)
                    va = vb
                if s1 == 129:
                    # row 255 from pair u=127 -> plane 0, slot 128
                    store_dma(out[0, :, NPAIR:NPAIR + 1, :],
                              sv[0:64, 128 - s0:129 - s0, :], True)

            # blocks of up to 4 units
            ui = 0
            while ui < len(units):
                emit_block(units[ui:ui + 4])
                ui += 4


_NC_CACHE = None


def _build_nc():
    global _NC_CACHE
    if _NC_CACHE is None:
        nc = bacc.Bacc("TRN2", target_bir_lowering=False, debug=False,
                       num_devices=B)
        with tile.TileContext(nc) as tc:
            _emit_kernel(tc)
        nc.compile()
        _NC_CACHE = nc
    return _NC_CACHE


def _in_maps(x, k_att_logits, dft_weight):
    x = np.asarray(x, dtype=np.float32)
    lg = np.asarray(k_att_logits, dtype=np.float32)
    dw = np.asarray(dft_weight, dtype=np.float32)

    # x -> bf16, parity-split rows, host-inserted zero pad columns
    xp = np.zeros((B, 2, CIN, NPAIR, SLOT), dtype=ml_dtypes.bfloat16)
    xv = x.reshape(B, CIN, NPAIR, 2, W).transpose(0, 3, 1, 2, 4)  # [b,j,c,t,w]
    xp[:, :, :, :, 1:1 + W] = xv.astype(ml_dtypes.bfloat16)

    # host-side gather: dense half-spectrum layout, kernel-major quadrants
    dftP = dw[:, _INV, :].reshape(KNUM, 2, 96, D2R, 2)   # [k, half, p, w, c]
    re = dftP[..., 0]             # [k, half, p, w]
    imneg = -dftP[..., 1]
    quad = np.stack([re[:, 0], re[:, 1], imneg[:, 0], imneg[:, 1]],
                    axis=1)       # [k, q, p, w]
    dd = np.ascontiguousarray(
        quad.transpose(2, 0, 1, 3).reshape(96, KNUM * 4 * D2R)
    ).astype(ml_dtypes.bfloat16)

    maps = []
    for b in range(B):
        maps.append({
            "x_in": np.ascontiguousarray(xp[b]),
            "lg_in": lg[b:b + 1],
            "dd_in": dd,
            "cs_in": _CS,
            "ab_in": _AB,
        })
    return maps


def _execute(x, k_att_logits, dft_weight, trace=False, **trace_kwargs):
    nc = _build_nc()
    res = run_bass_kernel_spmd(
        nc, _in_maps(x, k_att_logits, dft_weight),
        core_ids=list(range(B)), trace=trace, **trace_kwargs)
    out = np.empty((B, COUT, H, W), dtype=np.float32)
    for b in range(B):
        dev = res.results[b]["out"]      # [2, COUT, NPAIR+1, W] bf16
        out[b, :, 1::2, :] = dev[0, :, 1:NPAIR + 1, :].astype(np.float32)
        out[b, :, 0::2, :] = dev[1, :, 0:NPAIR, :].astype(np.float32)
    return out, res


def kernel(x, k_att_logits, dft_weight):
    out, _ = _execute(x, k_att_logits, dft_weight)
    return out.astype(np.float32)
